# revision 29
# baseline (speedup 1.0000x reference)
"""NVFP4 block-quantized linear layer (x @ w.T + bias) on 8 Trainium2 cores.

Reference semantics (reference.py): both activations and weights are
block-quantized along K (blocks of 16) to fp4-e2m1 with e4m3 scales
(scale = absmax/6, round-to-nearest), dequantized, then matmul with fp32
accumulation, cast to bf16, plus bf16 bias.

v2 device strategy (per core, 2-way M x 4-way N grid):
  - W quantized on VectorE, transposed via PE into a fully SBUF-resident
    wT [24 kc][128, 3072] bf16 (144KB/partition) -- no DRAM round trip,
    no mid-stream W DMAs.
  - x quantized per 128-row tile (VectorE), PE-transposed into xT tiles,
    pipelined one tile ahead of the matmul stream.
  - MM: for (mt, kc): stationary = xT[mt][:, kc] loaded ONCE (duplicate
    LDWEIGHTS instructions deleted post-hoc -- the PE retains the
    stationary operand), then 6 matmuls (nb=0..5, FD=512) accumulate
    out[mt, :] over kc into 6 PSUM banks (rotating over 7 banks).
  - psum -> bf16 on ScalarE, bf16 bias add on VectorE (matches the
    reference's double rounding), single 768KB out DMA per mt.
"""

import os
import numpy as np
import ml_dtypes

f32 = np.float32
bf16 = ml_dtypes.bfloat16

# ---------------------------------------------------------------------------
# problem geometry (hardcoded; harness calls kernel() with these full shapes)
B, T, K = 2, 4096, 3072
N = 12288
M = B * T                      # 8192
GRID_M, GRID_N = 2, 4          # 8 cores
M_CORE = M // GRID_M           # 4096
N_CORE = N // GRID_N           # 3072
NUM_CORES = GRID_M * GRID_N

CH1 = float(1.5 * 2**22)
RCP6 = float(f32(1.0) / f32(6.0))

_BUILT = None


# ---------------------------------------------------------------------------
def _register_custom_ops():
    """Register the two fp4-rounding custom DVE ops (idempotent)."""
    import concourse.dve_ops as dve_ops
    from concourse.dve_ops import DveOp, OPS, _SUB_OPCODE_FOR_NAME, _CUSTOM_DVE_ROW_BASE
    from concourse.dve_spec import (
        Spec, Src0, Src1, C0, C1, Zero, One, AluOp, Bin,
        maxx, minn, select, lower, _has_src1,
    )
    from concourse.dve_uop import DveOpSpec

    def _norm2(in0, in1):
        in0 = np.asarray(in0)
        in1 = np.asarray(in1)
        if in1.size != in0.size:
            in1 = np.broadcast_to(in1, in0.shape)
        return in0, np.ascontiguousarray(in1).reshape(in0.shape)

    def _ref_fp4_pre(in0, in1, s0, s1, imm2=None):
        in0, in1 = _norm2(in0, in1)
        m = (in0.astype(f32) * in1.astype(f32)).astype(f32)
        s2 = (m * m).astype(f32)
        ch = np.where(
            s2 < f32(4.0), f32(CH1),
            ((f32(1.0) + (s2 >= f32(16.0)).astype(f32)) * f32(1.5 * 2**23)).astype(f32),
        ).astype(f32)
        return (m + ch).astype(f32)

    def _ref_fp4_fin(in0, in1, s0, s1, imm2=None):
        in0, in1 = _norm2(in0, in1)
        qpre = np.ascontiguousarray(in0.astype(f32))
        pe = (qpre.view(np.uint32) & np.uint32(0x7F800000)).view(f32)
        d1 = (qpre - pe).astype(f32)
        q2 = ((d1 + d1).astype(f32) - pe).astype(f32)
        qc = np.maximum(np.minimum(q2, f32(12.0)), f32(-12.0))
        return (qc * in1.astype(f32)).astype(f32)

    def build_pre():
        SIXTEEN = C0 * C0
        Ch2x = C1 + C1
        m = Src0 * Src1
        s2 = m * m
        c2 = s2 >= SIXTEEN
        inner = (c2 + One) * Ch2x
        c1 = s2 < C0
        outer = select(c1, C1, inner)
        return Spec(body=m + outer, reference=_ref_fp4_pre)

    def build_fin():
        pe = Bin(AluOp.BITWISE_AND, Src0, C0)
        d1 = Src0 - pe
        q2 = (d1 + d1) - pe
        qc = maxx(minn(q2, C1), Zero - C1)
        return Spec(body=qc * Src1, reference=_ref_fp4_fin)

    def _ref_fp4_fin2(in0, in1, s0, s1, imm2=None):
        # like _ref_fp4_fin but in1 = s (not s/2); the x0.5 folded via imm2
        in0, in1 = _norm2(in0, in1)
        qpre = np.ascontiguousarray(in0.astype(f32))
        pe = (qpre.view(np.uint32) & np.uint32(0x7F800000)).view(f32)
        d1 = (qpre - pe).astype(f32)
        q2 = ((d1 + d1).astype(f32) - pe).astype(f32)
        qc = np.maximum(np.minimum(q2, f32(12.0)), f32(-12.0))
        return ((qc * f32(0.5)).astype(f32) * in1.astype(f32)).astype(f32)

    def build_fin2():
        from concourse.dve_spec import C2
        pe = Bin(AluOp.BITWISE_AND, Src0, C0)
        d1 = Src0 - pe
        q2 = (d1 + d1) - pe
        qc = maxx(minn(q2, C1), Zero - C1)
        return Spec(body=(qc * C2) * Src1, reference=_ref_fp4_fin2)

    def _ref_e4m3_rne(in0, in1, s0, s1, imm2=None):
        sraw = np.asarray(in0, dtype=f32)
        pe = (sraw.view(np.uint32) & np.uint32(0x7F800000)).view(f32)
        mt = np.maximum((pe * f32(1.5 * 2**20)).astype(f32), f32(1.5 * 2**14))
        return ((sraw + mt).astype(f32) - mt).astype(f32)

    def build_rne():
        from concourse.dve_spec import C2
        pe = Bin(AluOp.BITWISE_AND, Src0, C0)
        mt = maxx(pe * C1, C2)
        s = (Src0 + mt) - mt
        return Spec(body=s, reference=_ref_e4m3_rne)

    def register(name, spec):
        if name in _SUB_OPCODE_FOR_NAME:
            for op in OPS:
                if op.name == name:
                    return op
            raise RuntimeError(name)
        row = _CUSTOM_DVE_ROW_BASE + len(OPS)
        assert row < 0x20
        shas = {}
        for ver in ("v3", "v4"):
            try:
                uops = lower(spec, ver=ver)
            except Exception:
                continue
            shas[ver] = DveOpSpec(
                name=name, opcode=row, uops=uops, rd1_en=_has_src1(spec)
            ).sha(ver)
        op = DveOp(name, spec, subdim=False, uops_sha=shas)
        OPS.append(op)
        _SUB_OPCODE_FOR_NAME[name] = row
        dve_ops.CUSTOM_DVE_SPECS[name] = spec
        return op

    return (register("FP4_PRE_ANT", build_pre()),
            register("FP4_FIN_ANT", build_fin()),
            register("FP4_FIN2_ANT", build_fin2()),
            register("E4M3_RNE_ANT", build_rne()))


# ---------------------------------------------------------------------------
def _patch_tile_drain():
    """The TileContext tail drain attaches one sem-wait per live logical
    processor to a single SP Drain instruction; this walrus build caps sync
    waits per instruction at 2 ("Too many sync wait commands").  Split the
    overflow waits onto preceding single-wait SP nops (sound: all waits still
    complete before the post-drain all-engine barrier / sem reset)."""
    from concourse import tile as tile_mod
    import concourse.mybir as mybir
    from concourse.vector_clock import ScopedClock

    if getattr(tile_mod.TileContext, "_ant_drain_patched", False):
        return

    def _drain_and_barrier(self, tick_clock, wait_clock):
        nc = self.nc
        probe = nc.sync.nop()
        wait_clock.add_sem_waits(
            probe.ins, ScopedClock({None: tick_clock.global_clock})
        )
        si = probe.ins.sync_info
        waits = list(si.on_wait) if si is not None and si.on_wait else []
        if len(waits) > 1:
            probe.ins.sync_info = mybir.SyncInfo(
                on_wait=waits[:1],
                on_update=list(si.on_update) if si.on_update else [],
            )
            for w in waits[1:]:
                extra = nc.sync.nop()
                extra.ins.sync_info = mybir.SyncInfo(on_wait=[w], on_update=[])
        nc.sync.drain()

        nc.all_engine_barrier()
        assert self.sems is not None
        popped = nc._tile_sem_poison_stack.pop()
        assert popped is self._sem_poison
        nc.clear_and_free_semaphores(list(self.sems.allocated().values()))
        nc.all_engine_barrier()

    tile_mod.TileContext._drain_and_barrier = _drain_and_barrier
    tile_mod.TileContext._ant_drain_patched = True


def _split_excess_waits(nc, max_waits=1):
    """This walrus build rejects instructions carrying more than `max_waits`
    sem waits ("Too many sync wait commands").  Move overflow waits onto
    same-engine NoOp instructions inserted immediately before the offender —
    per-engine program order makes this semantically identical."""
    import concourse.mybir as mybir

    ctr = [0]
    for f in nc.m.functions:
        for blk in f.blocks:
            il = blk.instructions
            out = []
            changed = False
            for ins in il:
                si = ins.sync_info
                waits = list(si.on_wait) if si is not None and si.on_wait else []
                if len(waits) > max_waits:
                    changed = True
                    extra = waits[:-max_waits]
                    for i0 in range(0, len(extra), max_waits):
                        nop = mybir.InstNoOp(
                            name=f"I-waitsplit-{ctr[0]}", ins=[], outs=[])
                        ctr[0] += 1
                        nop.engine = ins.engine
                        nop.sync_info = mybir.SyncInfo(
                            on_wait=extra[i0:i0 + max_waits], on_update=[])
                        out.append(nop)
                    ins.sync_info = mybir.SyncInfo(
                        on_wait=waits[-max_waits:],
                        on_update=list(si.on_update) if si.on_update else [],
                    )
                out.append(ins)
            if changed:
                blk.instructions = out
    return ctr[0]


def _dedup_ldweights(nc):
    """Delete InstLdweights whose weights AP is byte-identical to the
    previous InstLdweights in the same block with no other InstLdweights in
    between: the PE retains the stationary operand across matmuls (verified
    on HW), so the reload is pure overhead (~101ns of PE issue each).
    Sync waits/updates on a deleted LDW migrate to the next instruction."""
    import concourse.mybir as mybir

    removed = 0
    for f in nc.m.functions:
        for blk in f.blocks:
            il = blk.instructions
            out = []
            last_ldw_key = None
            pending_sync = []
            for ins in il:
                nm = type(ins).__name__
                if nm == "InstLdweights":
                    key = (str(ins.ins[0]), bool(ins.is_transpose),
                           str(ins.perf_mode), str(ins.tile_position))
                    si = ins.sync_info
                    has_upd = si is not None and si.on_update
                    if key == last_ldw_key and not has_upd:
                        if si is not None and si.on_wait:
                            pending_sync.extend(list(si.on_wait))
                        removed += 1
                        continue
                    last_ldw_key = key
                if pending_sync:
                    si = ins.sync_info
                    ins.sync_info = mybir.SyncInfo(
                        on_wait=pending_sync + (
                            list(si.on_wait) if si is not None and si.on_wait else []),
                        on_update=(
                            list(si.on_update) if si is not None and si.on_update else []),
                    )
                    pending_sync = []
                out.append(ins)
            assert not pending_sync
            blk.instructions = out
    return removed


def build_nc(m_core=M_CORE, k=K, n_core=N_CORE, num_cores=NUM_CORES,
             debug=False, postprocess=True):
    """Build the per-core Bass program (SPMD: same program on every core)."""
    import concourse.bass as bass
    import concourse.mybir as mybir
    from concourse import tile
    from contextlib import ExitStack
    from collections import deque

    fp4_pre, fp4_fin, fp4_fin2, e4m3_rne = _register_custom_ops()
    _patch_tile_drain()

    KC = k // 128            # 24 k-chunks
    NR = n_core // 128       # 24 weight row-tiles
    MT = m_core // 128       # 32 x row-tiles
    NB = n_core // 512       # 6 psum column blocks
    HALF = k // 2            # 1536: quant processed per half tile
    KBH = HALF // 16         # 96 scale blocks per half

    nc = bass.Bass("TRN2", target_bir_lowering=False, debug=debug,
                   num_devices=num_cores)
    dt = mybir.dt
    Alu = mybir.AluOpType

    x_d = nc.dram_tensor("x", [m_core, k], dt.float32, kind="ExternalInput")
    w_d = nc.dram_tensor("w", [n_core, k], dt.float32, kind="ExternalInput")
    b_d = nc.dram_tensor("bias", [n_core], dt.bfloat16, kind="ExternalInput")
    id_d = nc.dram_tensor("ident", [128, 128], dt.bfloat16, kind="ExternalInput")
    out_d = nc.dram_tensor("out", [m_core, n_core], dt.bfloat16, kind="ExternalOutput")

    with tile.TileContext(nc) as tc, ExitStack() as ctx:
        cst = ctx.enter_context(tc.tile_pool(name="cst", bufs=1))
        wres = ctx.enter_context(tc.tile_pool(name="wres", bufs=1))
        xin = ctx.enter_context(tc.tile_pool(name="xin", bufs=2))
        blk = ctx.enter_context(tc.tile_pool(name="blk", bufs=1))
        xdqp = ctx.enter_context(tc.tile_pool(name="xdqp", bufs=2))
        xTp = ctx.enter_context(tc.tile_pool(name="xTp", bufs=3))
        obp = ctx.enter_context(tc.tile_pool(name="obp", bufs=2))
        obep = ctx.enter_context(tc.tile_pool(name="obep", bufs=1))
        ps_mm = ctx.enter_context(tc.tile_pool(name="ps_mm", bufs=1, space="PSUM"))
        ps_tr = ctx.enter_context(tc.tile_pool(name="ps_tr", bufs=2, space="PSUM"))

        ident = cst.tile([128, 128], dt.bfloat16, tag="ident")
        nc.sync.dma_start(out=ident[:, :], in_=id_d[:, :])
        # +inf per-partition scalar for FP4_FIN's exponent mask (an inf
        # *immediate* is not JSON-serializable through walrus)
        inf_t = cst.tile([128, 1], dt.float32, tag="inf")
        nc.vector.memset(inf_t[:, :], float("inf"))
        bias_t = cst.tile([128, n_core], dt.bfloat16, tag="bias")
        nc.sync.dma_start(
            out=bias_t[:, :],
            in_=b_d[:].unsqueeze(0).broadcast_to([128, n_core]),
        )

        # resident transposed weights: wT[kc] = [128 k, n_core]
        wT = [wres.tile([128, n_core], dt.bfloat16, tag=f"wT{kc}",
                        name=f"wT{kc}")
              for kc in range(KC)]

        def quant_half(src_d, r0, h, dst, inplace=True):
            """Quantize rows r0..r0+128, cols h*HALF..(h+1)*HALF of src_d
            into dst[:, h*HALF:(h+1)*HALF] (bf16 dequantized values)."""
            xt = xin.tile([128, HALF], dt.float32, tag="xin", name=f"xt{h}")
            nc.sync.dma_start(
                out=xt[:, :], in_=src_d[r0:r0 + 128, h * HALF:(h + 1) * HALF])
            x3 = xt[:, :].rearrange("p (b e) -> p b e", e=16)
            bm = blk.tile([128, KBH], dt.float32, tag="bm")
            nc.vector.tensor_reduce(
                bm[:, :], x3, axis=mybir.AxisListType.X, op=Alu.max,
                apply_absolute_value=True,
            )
            sraw = blk.tile([128, KBH], dt.float32, tag="sraw")
            nc.vector.tensor_scalar(
                sraw[:, :], bm[:, :], RCP6, float(2.0**-9), Alu.mult, Alu.max)
            # e4m3 RNE fused: s = (sraw + mt) - mt with
            # mt = max(2^expo(sraw) * 1.5*2^20, 1.5*2^14)
            s = blk.tile([128, KBH], dt.float32, tag="s")
            nc.vector._custom_dve(
                e4m3_rne, out=s[:, :], in0=sraw[:, :],
                s0=inf_t[:, 0:1], s1=float(1.5 * 2**20), imm2=float(1.5 * 2**14),
            )
            sh = blk.tile([128, KBH], dt.float32, tag="sh")
            nc.vector.tensor_scalar_mul(sh[:, :], s[:, :], 0.5)
            rinv = blk.tile([128, KBH], dt.float32, tag="rinv")
            nc.vector.reciprocal(rinv[:, :], s[:, :])

            # fp4 round stage 1 in place over the f32 input tile
            nc.vector._custom_dve(
                fp4_pre, out=x3, in0=x3,
                in1=rinv[:, :].unsqueeze(2).broadcast_to([128, KBH, 16]),
                s0=4.0, s1=CH1,
            )
            dst3 = dst[:, h * HALF:(h + 1) * HALF].rearrange(
                "p (b e) -> p b e", e=16)
            nc.vector._custom_dve(
                fp4_fin, out=dst3, in0=x3,
                in1=sh[:, :].unsqueeze(2).broadcast_to([128, KBH, 16]),
                s0=inf_t[:, 0:1], s1=12.0,
            )

        # ---- x pipeline helpers --------------------------------------------
        pending_T = deque()
        xT_tiles = {}

        def emit_x_quant(mt, xt_in_obp=False):
            xdq = xdqp.tile([128, k], dt.bfloat16, tag="xdq", name=f"xdq{mt}")
            for h in (0, 1):
                quant_half(x_d, mt * 128, h, xdq)
            if xt_in_obp:
                xTt = obp.tile([128, k], dt.bfloat16, tag="ob", name=f"xT{mt}")
            else:
                xTt = xTp.tile([128, k], dt.bfloat16, tag="xT", name=f"xT{mt}")
            xT_tiles[mt] = xTt
            for kc in range(KC):
                def run(xdq=xdq, xTt=xTt, kc=kc, mt=mt):
                    pst = ps_tr.tile([128, 128], dt.bfloat16, tag="tr",
                                     name=f"pstx{mt}_{kc}")
                    nc.tensor.transpose(
                        pst[:, :], xdq[:, kc * 128:(kc + 1) * 128], ident[:, :])
                    nc.scalar.copy(xTt[:, kc * 128:(kc + 1) * 128], pst[:, :])
                pending_T.append(run)

        # ---- W phase: quantize + transpose into resident wT; as each 512-col
        # block of wT completes, run mt0/mt1's full-K accumulation for that
        # block (fills the otherwise DVE-bound bubble with PE work).  The two
        # early x tiles (quantized into the idle out-staging buffers) are
        # emitted after W1 — they are only needed once 4 W tiles are done ----
        N_EARLY = 5
        wave_ctr = [0]

        def run_wave(mt, nb):
            pm = ps_mm.tile([128, 512], dt.float32,
                            tag=f"mm{wave_ctr[0] % 6}", name=f"pmw{mt}_{nb}")
            wave_ctr[0] += 1
            xTt = xT_tiles[mt]
            for kc in range(KC):
                nc.tensor.matmul(
                    pm[:, :], xTt[:, kc * 128:(kc + 1) * 128],
                    wT[kc][:, nb * 512:(nb + 1) * 512],
                    start=(kc == 0), stop=(kc == KC - 1),
                )
            obe = obep.tile([128, 512], dt.bfloat16, tag="obe",
                            name=f"obe{mt}_{nb}")
            nc.scalar.copy(obe[:, :], pm[:, :])
            nc.vector.tensor_tensor(
                obe[:, :], obe[:, :],
                bias_t[:, nb * 512:(nb + 1) * 512], Alu.add)
            nc.sync.dma_start(
                out=out_d[mt * 128:(mt + 1) * 128,
                          nb * 512:(nb + 1) * 512],
                in_=obe[:, :])

        wave = 0
        for nr in range(NR):
            wdq = xdqp.tile([128, k], dt.bfloat16, tag="xdq", name=f"wdq{nr}")
            for h in (0, 1):
                quant_half(w_d, nr * 128, h, wdq)
            if nr == 1:
                for mt in (0, 1):
                    emit_x_quant(mt)
                for _ in range(len(pending_T)):
                    pending_T.popleft()()
            if nr == 3:
                for mt in (2, 3):
                    emit_x_quant(mt, xt_in_obp=True)
                for _ in range(len(pending_T)):
                    pending_T.popleft()()
            if nr == 5:
                emit_x_quant(4)
                for _ in range(len(pending_T)):
                    pending_T.popleft()()
            for kc in range(KC):
                pst = ps_tr.tile([128, 128], dt.bfloat16, tag="tr",
                                 name=f"pstw{nr}_{kc}")
                nc.tensor.transpose(
                    pst[:, :], wdq[:, kc * 128:(kc + 1) * 128], ident[:, :])
                nc.scalar.copy(wT[kc][:, nr * 128:(nr + 1) * 128], pst[:, :])
            if nr % 4 == 3:
                run_wave(0, wave)
                run_wave(1, wave)
                if wave >= 1:
                    run_wave(2, wave - 1)
                    run_wave(3, wave - 1)
                if wave >= 2:
                    run_wave(4, wave - 2)
                wave += 1
        assert wave == NB
        run_wave(2, NB - 1)
        run_wave(3, NB - 1)
        run_wave(4, NB - 2)
        run_wave(4, NB - 1)
        for mt in range(N_EARLY):
            xT_tiles.pop(mt)

        # ---- main MM loop ---------------------------------------------------
        emit_x_quant(N_EARLY)
        for _ in range(len(pending_T)):
            pending_T.popleft()()
        for mt in range(N_EARLY, MT):
            if mt + 1 < MT:
                emit_x_quant(mt + 1)
            xTt = xT_tiles.pop(mt)
            psums = []
            for nb in range(NB):
                psums.append(ps_mm.tile(
                    [128, 512], dt.float32, tag=f"mm{nb}",
                    name=f"pmm{mt}_{nb}"))
            for kc in range(KC):
                # drain next tile's transposes in the back half of the block
                # (their DVE input is ready by then; earlier would stall PE)
                if kc >= KC // 2:
                    for _ in range(min(2, len(pending_T))):
                        pending_T.popleft()()
                lhs = xTt[:, kc * 128:(kc + 1) * 128]
                for nb in range(NB):
                    nc.tensor.matmul(
                        psums[nb][:, :], lhs,
                        wT[kc][:, nb * 512:(nb + 1) * 512],
                        start=(kc == 0), stop=(kc == KC - 1),
                    )
            assert not pending_T
            ob = obp.tile([128, n_core], dt.bfloat16, tag="ob", name=f"ob{mt}")
            for nb in range(NB):
                nc.scalar.copy(ob[:, nb * 512:(nb + 1) * 512], psums[nb][:, :])
            ob2 = ob[:, :]
            nc.vector.tensor_tensor(ob2, ob2, bias_t[:, :], Alu.add)
            nc.sync.dma_start(
                out=out_d[mt * 128:(mt + 1) * 128, :], in_=ob[:, :])

    if postprocess:
        n_dedup = _dedup_ldweights(nc)
        _split_excess_waits(nc)
        # Raw Bass skips the ISA-byte encoding pass (Bacc.compile runs it);
        # without it custom-DVE/extended insts ship empty .instr -> walrus
        # "ISA wrong length".
        mybir.codegen_inst_isa_subclasses(nc)
        if debug:
            print(f"dedup removed {n_dedup} InstLdweights")
    return nc


# ---------------------------------------------------------------------------
def _get_built():
    global _BUILT
    if _BUILT is None:
        _BUILT = build_nc()
    return _BUILT


def make_in_maps(x2, w, b):
    """Per-core input shards for the 2x4 (M x N) grid."""
    ident = np.eye(128, dtype=bf16)
    in_maps = []
    for c in range(NUM_CORES):
        mi, nj = divmod(c, GRID_N)
        in_maps.append({
            "x": x2[mi * M_CORE:(mi + 1) * M_CORE],
            "w": w[nj * N_CORE:(nj + 1) * N_CORE],
            "bias": b[nj * N_CORE:(nj + 1) * N_CORE],
            "ident": ident,
        })
    return in_maps


def kernel(x, weight, bias):
    """Full-input entry point: x [2,4096,3072] f32, weight [12288,3072] f32,
    bias [12288] bf16 -> out [2,4096,12288] bf16."""
    from concourse.bass_utils import run_bass_kernel_spmd

    nc = _get_built()
    x2 = np.ascontiguousarray(np.asarray(x, dtype=f32).reshape(M, K))
    w = np.ascontiguousarray(np.asarray(weight, dtype=f32))
    b = np.asarray(bias)
    if b.dtype != bf16:
        if b.dtype.itemsize == 2 and b.dtype.kind in "Vu":
            b = b.view(bf16)
        else:
            b = b.astype(bf16)

    in_maps = make_in_maps(x2, w, b)

    res = run_bass_kernel_spmd(nc, in_maps, list(range(NUM_CORES)))
    out = np.empty((M, N), dtype=bf16)
    for c in range(NUM_CORES):
        mi, nj = divmod(c, GRID_N)
        out[mi * M_CORE:(mi + 1) * M_CORE, nj * N_CORE:(nj + 1) * N_CORE] = (
            np.asarray(res.results[c]["out"]).astype(bf16, copy=False)
        )
    return out.reshape(B, T, N)


# revision 31
# speedup vs baseline: 1.0045x; 1.0045x over previous
"""NVFP4 block-quantized linear layer (x @ w.T + bias) on 8 Trainium2 cores.

Reference semantics (reference.py): both activations and weights are
block-quantized along K (blocks of 16) to fp4-e2m1 with e4m3 scales
(scale = absmax/6, round-to-nearest), dequantized, then matmul with fp32
accumulation, cast to bf16, plus bf16 bias.

v2 device strategy (per core, 2-way M x 4-way N grid):
  - W quantized on VectorE, transposed via PE into a fully SBUF-resident
    wT [24 kc][128, 3072] bf16 (144KB/partition) -- no DRAM round trip,
    no mid-stream W DMAs.
  - x quantized per 128-row tile (VectorE), PE-transposed into xT tiles,
    pipelined one tile ahead of the matmul stream.
  - MM: for (mt, kc): stationary = xT[mt][:, kc] loaded ONCE (duplicate
    LDWEIGHTS instructions deleted post-hoc -- the PE retains the
    stationary operand), then 6 matmuls (nb=0..5, FD=512) accumulate
    out[mt, :] over kc into 6 PSUM banks (rotating over 7 banks).
  - psum -> bf16 on ScalarE, bf16 bias add on VectorE (matches the
    reference's double rounding), single 768KB out DMA per mt.
"""

import os
import numpy as np
import ml_dtypes

f32 = np.float32
bf16 = ml_dtypes.bfloat16

# ---------------------------------------------------------------------------
# problem geometry (hardcoded; harness calls kernel() with these full shapes)
B, T, K = 2, 4096, 3072
N = 12288
M = B * T                      # 8192
GRID_M, GRID_N = 2, 4          # 8 cores
M_CORE = M // GRID_M           # 4096
N_CORE = N // GRID_N           # 3072
NUM_CORES = GRID_M * GRID_N

CH1 = float(1.5 * 2**22)
RCP6 = float(f32(1.0) / f32(6.0))

_BUILT = None


# ---------------------------------------------------------------------------
def _register_custom_ops():
    """Register the two fp4-rounding custom DVE ops (idempotent)."""
    import concourse.dve_ops as dve_ops
    from concourse.dve_ops import DveOp, OPS, _SUB_OPCODE_FOR_NAME, _CUSTOM_DVE_ROW_BASE
    from concourse.dve_spec import (
        Spec, Src0, Src1, C0, C1, Zero, One, AluOp, Bin,
        maxx, minn, select, lower, _has_src1,
    )
    from concourse.dve_uop import DveOpSpec

    def _norm2(in0, in1):
        in0 = np.asarray(in0)
        in1 = np.asarray(in1)
        if in1.size != in0.size:
            in1 = np.broadcast_to(in1, in0.shape)
        return in0, np.ascontiguousarray(in1).reshape(in0.shape)

    def _ref_fp4_pre(in0, in1, s0, s1, imm2=None):
        in0, in1 = _norm2(in0, in1)
        m = (in0.astype(f32) * in1.astype(f32)).astype(f32)
        s2 = (m * m).astype(f32)
        ch = np.where(
            s2 < f32(4.0), f32(CH1),
            ((f32(1.0) + (s2 >= f32(16.0)).astype(f32)) * f32(1.5 * 2**23)).astype(f32),
        ).astype(f32)
        return (m + ch).astype(f32)

    def _ref_fp4_fin(in0, in1, s0, s1, imm2=None):
        in0, in1 = _norm2(in0, in1)
        qpre = np.ascontiguousarray(in0.astype(f32))
        pe = (qpre.view(np.uint32) & np.uint32(0x7F800000)).view(f32)
        d1 = (qpre - pe).astype(f32)
        q2 = ((d1 + d1).astype(f32) - pe).astype(f32)
        qc = np.maximum(np.minimum(q2, f32(12.0)), f32(-12.0))
        return (qc * in1.astype(f32)).astype(f32)

    def build_pre():
        SIXTEEN = C0 * C0
        Ch2x = C1 + C1
        m = Src0 * Src1
        s2 = m * m
        c2 = s2 >= SIXTEEN
        inner = (c2 + One) * Ch2x
        c1 = s2 < C0
        outer = select(c1, C1, inner)
        return Spec(body=m + outer, reference=_ref_fp4_pre)

    def build_fin():
        pe = Bin(AluOp.BITWISE_AND, Src0, C0)
        d1 = Src0 - pe
        q2 = (d1 + d1) - pe
        qc = maxx(minn(q2, C1), Zero - C1)
        return Spec(body=qc * Src1, reference=_ref_fp4_fin)

    def _ref_fp4_fin2(in0, in1, s0, s1, imm2=None):
        # like _ref_fp4_fin but in1 = s (not s/2); the x0.5 folded via imm2
        in0, in1 = _norm2(in0, in1)
        qpre = np.ascontiguousarray(in0.astype(f32))
        pe = (qpre.view(np.uint32) & np.uint32(0x7F800000)).view(f32)
        d1 = (qpre - pe).astype(f32)
        q2 = ((d1 + d1).astype(f32) - pe).astype(f32)
        qc = np.maximum(np.minimum(q2, f32(12.0)), f32(-12.0))
        return ((qc * f32(0.5)).astype(f32) * in1.astype(f32)).astype(f32)

    def build_fin2():
        from concourse.dve_spec import C2
        pe = Bin(AluOp.BITWISE_AND, Src0, C0)
        d1 = Src0 - pe
        q2 = (d1 + d1) - pe
        qc = maxx(minn(q2, C1), Zero - C1)
        return Spec(body=(qc * C2) * Src1, reference=_ref_fp4_fin2)

    def _ref_e4m3_rne(in0, in1, s0, s1, imm2=None):
        sraw = np.asarray(in0, dtype=f32)
        pe = (sraw.view(np.uint32) & np.uint32(0x7F800000)).view(f32)
        mt = np.maximum((pe * f32(1.5 * 2**20)).astype(f32), f32(1.5 * 2**14))
        return ((sraw + mt).astype(f32) - mt).astype(f32)

    def build_rne():
        from concourse.dve_spec import C2
        pe = Bin(AluOp.BITWISE_AND, Src0, C0)
        mt = maxx(pe * C1, C2)
        s = (Src0 + mt) - mt
        return Spec(body=s, reference=_ref_e4m3_rne)

    def register(name, spec):
        if name in _SUB_OPCODE_FOR_NAME:
            for op in OPS:
                if op.name == name:
                    return op
            raise RuntimeError(name)
        row = _CUSTOM_DVE_ROW_BASE + len(OPS)
        assert row < 0x20
        shas = {}
        for ver in ("v3", "v4"):
            try:
                uops = lower(spec, ver=ver)
            except Exception:
                continue
            shas[ver] = DveOpSpec(
                name=name, opcode=row, uops=uops, rd1_en=_has_src1(spec)
            ).sha(ver)
        op = DveOp(name, spec, subdim=False, uops_sha=shas)
        OPS.append(op)
        _SUB_OPCODE_FOR_NAME[name] = row
        dve_ops.CUSTOM_DVE_SPECS[name] = spec
        return op

    return (register("FP4_PRE_ANT", build_pre()),
            register("FP4_FIN_ANT", build_fin()),
            register("FP4_FIN2_ANT", build_fin2()),
            register("E4M3_RNE_ANT", build_rne()))


# ---------------------------------------------------------------------------
def _patch_tile_drain():
    """The TileContext tail drain attaches one sem-wait per live logical
    processor to a single SP Drain instruction; this walrus build caps sync
    waits per instruction at 2 ("Too many sync wait commands").  Split the
    overflow waits onto preceding single-wait SP nops (sound: all waits still
    complete before the post-drain all-engine barrier / sem reset)."""
    from concourse import tile as tile_mod
    import concourse.mybir as mybir
    from concourse.vector_clock import ScopedClock

    if getattr(tile_mod.TileContext, "_ant_drain_patched", False):
        return

    def _drain_and_barrier(self, tick_clock, wait_clock):
        nc = self.nc
        probe = nc.sync.nop()
        wait_clock.add_sem_waits(
            probe.ins, ScopedClock({None: tick_clock.global_clock})
        )
        si = probe.ins.sync_info
        waits = list(si.on_wait) if si is not None and si.on_wait else []
        if len(waits) > 1:
            probe.ins.sync_info = mybir.SyncInfo(
                on_wait=waits[:1],
                on_update=list(si.on_update) if si.on_update else [],
            )
            for w in waits[1:]:
                extra = nc.sync.nop()
                extra.ins.sync_info = mybir.SyncInfo(on_wait=[w], on_update=[])
        nc.sync.drain()

        nc.all_engine_barrier()
        assert self.sems is not None
        popped = nc._tile_sem_poison_stack.pop()
        assert popped is self._sem_poison
        nc.clear_and_free_semaphores(list(self.sems.allocated().values()))
        nc.all_engine_barrier()

    tile_mod.TileContext._drain_and_barrier = _drain_and_barrier
    tile_mod.TileContext._ant_drain_patched = True


def _split_excess_waits(nc, max_waits=1):
    """This walrus build rejects instructions carrying more than `max_waits`
    sem waits ("Too many sync wait commands").  Move overflow waits onto
    same-engine NoOp instructions inserted immediately before the offender —
    per-engine program order makes this semantically identical."""
    import concourse.mybir as mybir

    ctr = [0]
    for f in nc.m.functions:
        for blk in f.blocks:
            il = blk.instructions
            out = []
            changed = False
            for ins in il:
                si = ins.sync_info
                waits = list(si.on_wait) if si is not None and si.on_wait else []
                if len(waits) > max_waits:
                    changed = True
                    extra = waits[:-max_waits]
                    for i0 in range(0, len(extra), max_waits):
                        nop = mybir.InstNoOp(
                            name=f"I-waitsplit-{ctr[0]}", ins=[], outs=[])
                        ctr[0] += 1
                        nop.engine = ins.engine
                        nop.sync_info = mybir.SyncInfo(
                            on_wait=extra[i0:i0 + max_waits], on_update=[])
                        out.append(nop)
                    ins.sync_info = mybir.SyncInfo(
                        on_wait=waits[-max_waits:],
                        on_update=list(si.on_update) if si.on_update else [],
                    )
                out.append(ins)
            if changed:
                blk.instructions = out
    return ctr[0]


def _dedup_ldweights(nc):
    """Delete InstLdweights whose weights AP is byte-identical to the
    previous InstLdweights in the same block with no other InstLdweights in
    between: the PE retains the stationary operand across matmuls (verified
    on HW), so the reload is pure overhead (~101ns of PE issue each).
    Sync waits/updates on a deleted LDW migrate to the next instruction."""
    import concourse.mybir as mybir

    removed = 0
    for f in nc.m.functions:
        for blk in f.blocks:
            il = blk.instructions
            out = []
            last_ldw_key = None
            pending_sync = []
            for ins in il:
                nm = type(ins).__name__
                if nm == "InstLdweights":
                    key = (str(ins.ins[0]), bool(ins.is_transpose),
                           str(ins.perf_mode), str(ins.tile_position))
                    si = ins.sync_info
                    has_upd = si is not None and si.on_update
                    if key == last_ldw_key and not has_upd:
                        if si is not None and si.on_wait:
                            pending_sync.extend(list(si.on_wait))
                        removed += 1
                        continue
                    last_ldw_key = key
                if pending_sync:
                    si = ins.sync_info
                    ins.sync_info = mybir.SyncInfo(
                        on_wait=pending_sync + (
                            list(si.on_wait) if si is not None and si.on_wait else []),
                        on_update=(
                            list(si.on_update) if si is not None and si.on_update else []),
                    )
                    pending_sync = []
                out.append(ins)
            assert not pending_sync
            blk.instructions = out
    return removed


def build_nc(m_core=M_CORE, k=K, n_core=N_CORE, num_cores=NUM_CORES,
             debug=False, postprocess=True):
    """Build the per-core Bass program (SPMD: same program on every core)."""
    import concourse.bass as bass
    import concourse.mybir as mybir
    from concourse import tile
    from contextlib import ExitStack
    from collections import deque

    fp4_pre, fp4_fin, fp4_fin2, e4m3_rne = _register_custom_ops()
    _patch_tile_drain()

    KC = k // 128            # 24 k-chunks
    NR = n_core // 128       # 24 weight row-tiles
    MT = m_core // 128       # 32 x row-tiles
    NB = n_core // 512       # 6 psum column blocks
    HALF = k // 2            # 1536: quant processed per half tile
    KBH = HALF // 16         # 96 scale blocks per half

    nc = bass.Bass("TRN2", target_bir_lowering=False, debug=debug,
                   num_devices=num_cores)
    dt = mybir.dt
    Alu = mybir.AluOpType

    x_d = nc.dram_tensor("x", [m_core, k], dt.float32, kind="ExternalInput")
    w_d = nc.dram_tensor("w", [n_core, k], dt.float32, kind="ExternalInput")
    b_d = nc.dram_tensor("bias", [n_core], dt.bfloat16, kind="ExternalInput")
    id_d = nc.dram_tensor("ident", [128, 128], dt.bfloat16, kind="ExternalInput")
    out_d = nc.dram_tensor("out", [m_core, n_core], dt.bfloat16, kind="ExternalOutput")

    with tile.TileContext(nc) as tc, ExitStack() as ctx:
        cst = ctx.enter_context(tc.tile_pool(name="cst", bufs=1))
        wres = ctx.enter_context(tc.tile_pool(name="wres", bufs=1))
        xin = ctx.enter_context(tc.tile_pool(name="xin", bufs=2))
        blk = ctx.enter_context(tc.tile_pool(name="blk", bufs=1))
        xdqp = ctx.enter_context(tc.tile_pool(name="xdqp", bufs=2))
        xTp = ctx.enter_context(tc.tile_pool(name="xTp", bufs=2))
        obp = ctx.enter_context(tc.tile_pool(name="obp", bufs=2))
        obep = ctx.enter_context(tc.tile_pool(name="obep", bufs=2))
        ps_mm = ctx.enter_context(tc.tile_pool(name="ps_mm", bufs=1, space="PSUM"))
        ps_tr = ctx.enter_context(tc.tile_pool(name="ps_tr", bufs=2, space="PSUM"))

        ident = cst.tile([128, 128], dt.bfloat16, tag="ident")
        nc.sync.dma_start(out=ident[:, :], in_=id_d[:, :])
        # +inf per-partition scalar for FP4_FIN's exponent mask (an inf
        # *immediate* is not JSON-serializable through walrus)
        inf_t = cst.tile([128, 1], dt.float32, tag="inf")
        nc.vector.memset(inf_t[:, :], float("inf"))
        bias_t = cst.tile([128, n_core], dt.bfloat16, tag="bias")
        nc.sync.dma_start(
            out=bias_t[:, :],
            in_=b_d[:].unsqueeze(0).broadcast_to([128, n_core]),
        )

        # resident transposed weights: wT[kc] = [128 k, n_core]
        wT = [wres.tile([128, n_core], dt.bfloat16, tag=f"wT{kc}",
                        name=f"wT{kc}")
              for kc in range(KC)]

        def quant_half(src_d, r0, h, dst, inplace=True):
            """Quantize rows r0..r0+128, cols h*HALF..(h+1)*HALF of src_d
            into dst[:, h*HALF:(h+1)*HALF] (bf16 dequantized values)."""
            xt = xin.tile([128, HALF], dt.float32, tag="xin", name=f"xt{h}")
            nc.sync.dma_start(
                out=xt[:, :], in_=src_d[r0:r0 + 128, h * HALF:(h + 1) * HALF])
            x3 = xt[:, :].rearrange("p (b e) -> p b e", e=16)
            bm = blk.tile([128, KBH], dt.float32, tag="bm")
            nc.vector.tensor_reduce(
                bm[:, :], x3, axis=mybir.AxisListType.X, op=Alu.max,
                apply_absolute_value=True,
            )
            sraw = blk.tile([128, KBH], dt.float32, tag="sraw")
            nc.vector.tensor_scalar(
                sraw[:, :], bm[:, :], RCP6, float(2.0**-9), Alu.mult, Alu.max)
            # e4m3 RNE fused: s = (sraw + mt) - mt with
            # mt = max(2^expo(sraw) * 1.5*2^20, 1.5*2^14)
            s = blk.tile([128, KBH], dt.float32, tag="s")
            nc.vector._custom_dve(
                e4m3_rne, out=s[:, :], in0=sraw[:, :],
                s0=inf_t[:, 0:1], s1=float(1.5 * 2**20), imm2=float(1.5 * 2**14),
            )
            sh = blk.tile([128, KBH], dt.float32, tag="sh")
            nc.vector.tensor_scalar_mul(sh[:, :], s[:, :], 0.5)
            rinv = blk.tile([128, KBH], dt.float32, tag="rinv")
            nc.vector.reciprocal(rinv[:, :], s[:, :])

            # fp4 round stage 1 in place over the f32 input tile
            nc.vector._custom_dve(
                fp4_pre, out=x3, in0=x3,
                in1=rinv[:, :].unsqueeze(2).broadcast_to([128, KBH, 16]),
                s0=4.0, s1=CH1,
            )
            dst3 = dst[:, h * HALF:(h + 1) * HALF].rearrange(
                "p (b e) -> p b e", e=16)
            nc.vector._custom_dve(
                fp4_fin, out=dst3, in0=x3,
                in1=sh[:, :].unsqueeze(2).broadcast_to([128, KBH, 16]),
                s0=inf_t[:, 0:1], s1=12.0,
            )

        # ---- x pipeline helpers --------------------------------------------
        pending_T = deque()
        xT_tiles = {}

        def emit_x_quant(mt, xt_in_obp=False):
            xdq = xdqp.tile([128, k], dt.bfloat16, tag="xdq", name=f"xdq{mt}")
            for h in (0, 1):
                quant_half(x_d, mt * 128, h, xdq)
            if xt_in_obp:
                xTt = obp.tile([128, k], dt.bfloat16, tag="ob", name=f"xT{mt}")
            else:
                xTt = xTp.tile([128, k], dt.bfloat16, tag="xT", name=f"xT{mt}")
            xT_tiles[mt] = xTt
            for kc in range(KC):
                def run(xdq=xdq, xTt=xTt, kc=kc, mt=mt):
                    pst = ps_tr.tile([128, 128], dt.bfloat16, tag="tr",
                                     name=f"pstx{mt}_{kc}")
                    nc.tensor.transpose(
                        pst[:, :], xdq[:, kc * 128:(kc + 1) * 128], ident[:, :])
                    nc.scalar.copy(xTt[:, kc * 128:(kc + 1) * 128], pst[:, :])
                pending_T.append(run)

        # ---- W phase: quantize + transpose into resident wT; as each 512-col
        # block of wT completes, run mt0/mt1's full-K accumulation for that
        # block (fills the otherwise DVE-bound bubble with PE work).  The two
        # early x tiles (quantized into the idle out-staging buffers) are
        # emitted after W1 — they are only needed once 4 W tiles are done ----
        N_EARLY = 4
        wave_ctr = [0]

        def run_wave(mt, nb):
            pm = ps_mm.tile([128, 512], dt.float32,
                            tag=f"mm{wave_ctr[0] % 6}", name=f"pmw{mt}_{nb}")
            wave_ctr[0] += 1
            xTt = xT_tiles[mt]
            for kc in range(KC):
                nc.tensor.matmul(
                    pm[:, :], xTt[:, kc * 128:(kc + 1) * 128],
                    wT[kc][:, nb * 512:(nb + 1) * 512],
                    start=(kc == 0), stop=(kc == KC - 1),
                )
            obe = obep.tile([128, 512], dt.bfloat16, tag="obe",
                            name=f"obe{mt}_{nb}")
            nc.scalar.copy(obe[:, :], pm[:, :])
            nc.vector.tensor_tensor(
                obe[:, :], obe[:, :],
                bias_t[:, nb * 512:(nb + 1) * 512], Alu.add)
            nc.sync.dma_start(
                out=out_d[mt * 128:(mt + 1) * 128,
                          nb * 512:(nb + 1) * 512],
                in_=obe[:, :])

        wave = 0
        for nr in range(NR):
            wdq = xdqp.tile([128, k], dt.bfloat16, tag="xdq", name=f"wdq{nr}")
            for h in (0, 1):
                quant_half(w_d, nr * 128, h, wdq)
            if nr == 1:
                for mt in (0, 1):
                    emit_x_quant(mt)
                for _ in range(len(pending_T)):
                    pending_T.popleft()()
            if nr == 3:
                for mt in (2, 3):
                    emit_x_quant(mt, xt_in_obp=True)
                for _ in range(len(pending_T)):
                    pending_T.popleft()()
            for kc in range(KC):
                pst = ps_tr.tile([128, 128], dt.bfloat16, tag="tr",
                                 name=f"pstw{nr}_{kc}")
                nc.tensor.transpose(
                    pst[:, :], wdq[:, kc * 128:(kc + 1) * 128], ident[:, :])
                nc.scalar.copy(wT[kc][:, nr * 128:(nr + 1) * 128], pst[:, :])
            if nr % 4 == 3:
                run_wave(0, wave)
                run_wave(1, wave)
                if wave >= 1:
                    run_wave(2, wave - 1)
                    run_wave(3, wave - 1)
                wave += 1
        assert wave == NB
        # x4's quant chain (DVE) overlaps the final lagged waves (PE)
        emit_x_quant(N_EARLY)
        run_wave(2, NB - 1)
        run_wave(3, NB - 1)
        for mt in range(N_EARLY):
            xT_tiles.pop(mt)

        # ---- main MM loop ---------------------------------------------------
        for _ in range(len(pending_T)):
            pending_T.popleft()()
        for mt in range(N_EARLY, MT):
            if mt + 1 < MT:
                emit_x_quant(mt + 1)
            xTt = xT_tiles.pop(mt)
            psums = []
            for nb in range(NB):
                psums.append(ps_mm.tile(
                    [128, 512], dt.float32, tag=f"mm{nb}",
                    name=f"pmm{mt}_{nb}"))
            for kc in range(KC):
                # drain next tile's transposes in the back half of the block
                # (their DVE input is ready by then; earlier would stall PE)
                if kc >= KC // 2:
                    for _ in range(min(2, len(pending_T))):
                        pending_T.popleft()()
                lhs = xTt[:, kc * 128:(kc + 1) * 128]
                for nb in range(NB):
                    nc.tensor.matmul(
                        psums[nb][:, :], lhs,
                        wT[kc][:, nb * 512:(nb + 1) * 512],
                        start=(kc == 0), stop=(kc == KC - 1),
                    )
            assert not pending_T
            ob = obp.tile([128, n_core], dt.bfloat16, tag="ob", name=f"ob{mt}")
            for nb in range(NB):
                nc.scalar.copy(ob[:, nb * 512:(nb + 1) * 512], psums[nb][:, :])
            ob2 = ob[:, :]
            nc.vector.tensor_tensor(ob2, ob2, bias_t[:, :], Alu.add)
            nc.sync.dma_start(
                out=out_d[mt * 128:(mt + 1) * 128, :], in_=ob[:, :])

    if postprocess:
        n_dedup = _dedup_ldweights(nc)
        _split_excess_waits(nc)
        # Raw Bass skips the ISA-byte encoding pass (Bacc.compile runs it);
        # without it custom-DVE/extended insts ship empty .instr -> walrus
        # "ISA wrong length".
        mybir.codegen_inst_isa_subclasses(nc)
        if debug:
            print(f"dedup removed {n_dedup} InstLdweights")
    return nc


# ---------------------------------------------------------------------------
def _get_built():
    global _BUILT
    if _BUILT is None:
        _BUILT = build_nc()
    return _BUILT


def make_in_maps(x2, w, b):
    """Per-core input shards for the 2x4 (M x N) grid."""
    ident = np.eye(128, dtype=bf16)
    in_maps = []
    for c in range(NUM_CORES):
        mi, nj = divmod(c, GRID_N)
        in_maps.append({
            "x": x2[mi * M_CORE:(mi + 1) * M_CORE],
            "w": w[nj * N_CORE:(nj + 1) * N_CORE],
            "bias": b[nj * N_CORE:(nj + 1) * N_CORE],
            "ident": ident,
        })
    return in_maps


def kernel(x, weight, bias):
    """Full-input entry point: x [2,4096,3072] f32, weight [12288,3072] f32,
    bias [12288] bf16 -> out [2,4096,12288] bf16."""
    from concourse.bass_utils import run_bass_kernel_spmd

    nc = _get_built()
    x2 = np.ascontiguousarray(np.asarray(x, dtype=f32).reshape(M, K))
    w = np.ascontiguousarray(np.asarray(weight, dtype=f32))
    b = np.asarray(bias)
    if b.dtype != bf16:
        if b.dtype.itemsize == 2 and b.dtype.kind in "Vu":
            b = b.view(bf16)
        else:
            b = b.astype(bf16)

    in_maps = make_in_maps(x2, w, b)

    res = run_bass_kernel_spmd(nc, in_maps, list(range(NUM_CORES)))
    out = np.empty((M, N), dtype=bf16)
    for c in range(NUM_CORES):
        mi, nj = divmod(c, GRID_N)
        out[mi * M_CORE:(mi + 1) * M_CORE, nj * N_CORE:(nj + 1) * N_CORE] = (
            np.asarray(res.results[c]["out"]).astype(bf16, copy=False)
        )
    return out.reshape(B, T, N)
